# revision 1
# baseline (speedup 1.0000x reference)
"""Trainium2 Bass kernel: 4-layer MLP (784-512-512-512-10) + log_softmax.

Data-parallel over 8 NeuronCores: batch 65536 is split into 8 shards of
8192 rows; the ~1M-param weights are replicated on every core.

Layout: activations live on-chip transposed ([features, batch]) so every
layer's matmul is `out[of, nb] += W_lT[if, of].T @ h[if, nb]` with the
feature chunks on partitions.  Layers 1-3 run in fp8e4 with DoubleRow
(two 128-feature contraction chunks per matmul, fp32 PSUM accumulate);
layer 4 runs in bf16 with the batch flipped onto PSUM partitions so the
softmax reduces along the free dim.  Batch is processed in superchunks
of 1024 rows: each output-feature chunk accumulates two 512-column
halves into one 2-bank PSUM tile so a single 1024-wide op applies
bias+ReLU (all on ScalarE, the faster PSUM drainer — measured best).  log_softmax skips the max-subtraction (logits are small
enough for fp32 exp), accumulates sum(exp) via the Exp activation's
accum_out, and applies a batched Ln + broadcast-subtract epilogue, split
so most of it hides under the last superchunk's matmuls.  A short dummy-
matmul warm-up keeps the PE HAM clock gate at 2.4 GHz through the
initial DMA wait.

Measured on 8 axon trn2 cores: ~169-171 us HW exec per NEFF
(~212 us when the device sits in its throttled power state),
absmax 1.39e-2 / scale-relative 5.6e-3 / max-relative 6.1e-3 vs the
fp32 jax reference (fp8 quantization dominates the error; the all-bf16
variant measured 257 us at 4.3e-4 max-relative).
"""

from contextlib import ExitStack

import ml_dtypes
import numpy as np

import concourse.bass as bass  # noqa: F401  (registers AP machinery)
from concourse import bacc, mybir
from concourse.bass_utils import run_bass_kernel_spmd
from concourse.tile import TileContext

BF16 = mybir.dt.bfloat16
FP32 = mybir.dt.float32
FP8 = mybir.dt.float8e4

N_CORES = 8
B = 65536
D0, H, C = 784, 512, 10
BC = B // N_CORES            # 8192 rows per core
NB = 512                     # matmul moving free dim / PSUM bank width
HB = 2                       # batch halves sharing one PSUM group
SNB = NB * HB                # 1024-row superchunk
NCHUNK = BC // SNB           # 8 superchunks
K0F = D0 // 128              # 6 full 128-row contraction chunks in layer 1
K0R = D0 - K0F * 128         # 16 remainder rows
KH = H // 128                # 4 contraction chunks for hidden layers
NRG = BC // 128              # 64 row-groups of 128 rows per core

_CACHED_NC = None


def build_nc():
    nc = bacc.Bacc(
        "TRN2",
        target_bir_lowering=False,
        debug=False,
        enable_asserts=False,
        num_devices=N_CORES,
    )
    xt_d = nc.declare_dram_parameter("xt", [D0, BC], FP8, isOutput=False)
    w1_d = nc.declare_dram_parameter("w1t", [D0, H], FP8, isOutput=False)
    w2_d = nc.declare_dram_parameter("w2t", [H, H], FP8, isOutput=False)
    w3_d = nc.declare_dram_parameter("w3t", [H, H], FP8, isOutput=False)
    w4_d = nc.declare_dram_parameter("w4t", [H, C], BF16, isOutput=False)
    b1_d = nc.declare_dram_parameter("b1", [H], FP32, isOutput=False)
    b2_d = nc.declare_dram_parameter("b2", [H], FP32, isOutput=False)
    b3_d = nc.declare_dram_parameter("b3", [H], FP32, isOutput=False)
    b4_d = nc.declare_dram_parameter("b4r", [128, C], FP32, isOutput=False)
    out_d = nc.declare_dram_parameter("out", [BC, C], FP32, isOutput=True)

    expf = mybir.ActivationFunctionType.Exp
    reluf = mybir.ActivationFunctionType.Relu
    lnf = mybir.ActivationFunctionType.Ln
    add_op = mybir.AluOpType.add
    max_op = mybir.AluOpType.max
    sub_op = mybir.AluOpType.subtract
    drow = mybir.MatmulPerfMode.DoubleRow

    with TileContext(nc) as tc, ExitStack() as ctx:
        consts = ctx.enter_context(tc.tile_pool(name="consts", bufs=1))
        xpool = ctx.enter_context(tc.tile_pool(name="xp", bufs=6))
        hpool = ctx.enter_context(tc.tile_pool(name="hp", bufs=3))
        spool = ctx.enter_context(tc.tile_pool(name="sp", bufs=4))
        pbig = ctx.enter_context(tc.tile_pool(name="pbig", bufs=3, space="PSUM"))
        psml = ctx.enter_context(tc.tile_pool(name="psml", bufs=2, space="PSUM"))

        # Resident weights/biases, loaded once (ScalarE DMA queue so the
        # SP queue is free for the first x superchunk).
        w1 = consts.tile([128, K0F + 1, H], FP8, tag="w1")
        for k in range(K0F):
            nc.scalar.dma_start(w1[:, k, :], w1_d[k * 128 : (k + 1) * 128, :])
        nc.scalar.dma_start(w1[0:K0R, K0F, :], w1_d[K0F * 128 : D0, :])
        w2 = consts.tile([128, KH, H], FP8, tag="w2")
        nc.scalar.dma_start(w2[:], w2_d.rearrange("(o p) n -> p o n", p=128))
        w3 = consts.tile([128, KH, H], FP8, tag="w3")
        nc.scalar.dma_start(w3[:], w3_d.rearrange("(o p) n -> p o n", p=128))
        w4 = consts.tile([128, KH, C], BF16, tag="w4")
        nc.scalar.dma_start(w4[:], w4_d.rearrange("(o p) n -> p o n", p=128))
        b1s = consts.tile([128, KH], FP32, tag="b1")
        nc.scalar.dma_start(b1s[:], b1_d.rearrange("(o p) -> p o", p=128))
        b2s = consts.tile([128, KH], FP32, tag="b2")
        nc.scalar.dma_start(b2s[:], b2_d.rearrange("(o p) -> p o", p=128))
        b3s = consts.tile([128, KH], FP32, tag="b3")
        nc.scalar.dma_start(b3s[:], b3_d.rearrange("(o p) -> p o", p=128))
        b4s = consts.tile([128, C], FP32, tag="b4")
        nc.scalar.dma_start(b4s[:], b4_d[:])

        # PE warm-up: ~3.5us of dummy matmuls during the initial DMA wait
        # so the HAM clock gate is at 2.4 GHz when real work arrives.
        warm = consts.tile([128, NB], FP8, tag="warm")
        nc.vector.memset(warm[:], 1.0)
        psw = pbig.tile([128, HB, NB], FP32, tag="ps", name="ps_warm")
        for i in range(26):
            nc.tensor.matmul(
                psw[:, i % 2, :], lhsT=warm[:, 0:128], rhs=warm[:],
                start=(i < 2), stop=(i >= 24),
            )

        # Persistent softmax state for all 64 row-groups.
        logits_all = consts.tile([128, NRG, C], FP32, tag="logits_all")
        esum_all = consts.tile([128, NRG], FP32, tag="esum_all")
        lns_all = consts.tile([128, NRG], FP32, tag="lns_all")
        obuf = consts.tile([128, NRG, C], FP32, tag="obuf")

        def softmax_epilogue(rg0, rg1):
            # out = logits - ln(sum(exp(logits))) for row-groups [rg0, rg1)
            n = rg1 - rg0
            nc.scalar.activation(lns_all[:, rg0:rg1], esum_all[:, rg0:rg1], lnf)
            nc.vector.tensor_tensor(
                obuf[:, rg0:rg1, :], logits_all[:, rg0:rg1, :],
                lns_all[:, rg0:rg1, None].to_broadcast((128, n, C)), sub_op,
            )
            nc.sync.dma_start(
                out_d[rg0 * 128 : rg1 * 128, :].rearrange("(o p) n -> p o n", p=128),
                obuf[:, rg0:rg1, :],
            )

        for sc in range(NCHUNK):
            b0 = sc * SNB
            xt = xpool.tile([128, K0F + 1, SNB], FP8, tag="xt")
            for k in range(K0F):
                nc.sync.dma_start(
                    xt[:, k, :], xt_d[k * 128 : (k + 1) * 128, b0 : b0 + SNB]
                )
            nc.sync.dma_start(xt[0:K0R, K0F, :], xt_d[K0F * 128 : D0, b0 : b0 + SNB])

            # Layer 1 [784 -> 512]: fp8 DoubleRow, K=16 remainder plain fp8.
            # Both batch halves accumulate into one 2-bank PSUM tile so a
            # single DVE op applies bias+ReLU to the full superchunk row.
            h1p = [
                hpool.tile([128, 2, HB, NB], FP8, tag=f"h1p_{j}", name=f"h1p_{j}")
                for j in range(KH // 2)
            ]
            for m in range(KH):
                ps = pbig.tile([128, HB, NB], FP32, tag="ps")
                ms = slice(m * 128, (m + 1) * 128)
                # DoubleRow pairs first, K=16 remainder last: the
                # group opens with a plain DR matmul (no mode-switch stall)
                # and superchunk 0's first matmuls only need k-chunk 0/1.
                for k in range(0, K0F, 2):
                    for hb in range(HB):
                        nc.tensor.matmul(
                            ps[:, hb, :], lhsT=w1[:, k : k + 2, ms],
                            rhs=xt[:, k : k + 2, hb * NB : (hb + 1) * NB],
                            start=(k == 0), stop=False, perf_mode=drow,
                        )
                for hb in range(HB):
                    nc.tensor.matmul(
                        ps[:, hb, :], lhsT=w1[0:K0R, K0F, ms],
                        rhs=xt[0:K0R, K0F, hb * NB : (hb + 1) * NB],
                        start=False, stop=True, perf_mode=None,
                    )
                nc.scalar.activation(
                    h1p[m // 2][:, m % 2, :, :], ps[:], reluf,
                    bias=b1s[:, m : m + 1],
                )

            # Layer 2 [512 -> 512]: fp8 DoubleRow over feature-chunk pairs.
            h2p = [
                hpool.tile([128, 2, HB, NB], FP8, tag=f"h2p_{j}", name=f"h2p_{j}")
                for j in range(KH // 2)
            ]
            for m in range(KH):
                ps = pbig.tile([128, HB, NB], FP32, tag="ps")
                ms = slice(m * 128, (m + 1) * 128)
                for j in range(KH // 2):
                    for hb in range(HB):
                        nc.tensor.matmul(
                            ps[:, hb, :], lhsT=w2[:, 2 * j : 2 * j + 2, ms],
                            rhs=h1p[j][:, :, hb, :],
                            start=(j == 0), stop=(j == KH // 2 - 1),
                            perf_mode=drow,
                        )
                nc.scalar.activation(
                    h2p[m // 2][:, m % 2, :, :], ps[:], reluf,
                    bias=b2s[:, m : m + 1],
                )

            # Layer 3 [512 -> 512]: fp8 DoubleRow in, bf16 out (layer-4 lhsT).
            h3 = [
                hpool.tile([128, HB, NB], BF16, tag=f"h3_{m}", name=f"h3_{m}")
                for m in range(KH)
            ]
            for m in range(KH):
                ps = pbig.tile([128, HB, NB], FP32, tag="ps")
                ms = slice(m * 128, (m + 1) * 128)
                for j in range(KH // 2):
                    for hb in range(HB):
                        nc.tensor.matmul(
                            ps[:, hb, :], lhsT=w3[:, 2 * j : 2 * j + 2, ms],
                            rhs=h2p[j][:, :, hb, :],
                            start=(j == 0), stop=(j == KH // 2 - 1),
                            perf_mode=drow,
                        )
                nc.scalar.activation(h3[m][:], ps[:], reluf, bias=b3s[:, m : m + 1])

            # Layer 4 [512 -> 10], bf16, output flipped to [batch, 10].
            # All 8 row-groups of the superchunk accumulate into one PSUM
            # bank, so bias-add / exp / sum(exp) run as 3 batched ops.
            MG = SNB // 128
            rg0 = sc * MG
            ps4 = psml.tile([128, MG, C], FP32, tag="ps4")
            for hb in range(HB):
                for mm in range(NB // 128):
                    r = hb * (NB // 128) + mm
                    ms = slice(mm * 128, (mm + 1) * 128)
                    for k in range(KH):
                        nc.tensor.matmul(
                            ps4[:, r, :], lhsT=h3[k][:, hb, ms], rhs=w4[:, k, :],
                            start=(k == 0), stop=(k == KH - 1),
                        )
            lg = logits_all[:, rg0 : rg0 + MG, :]
            nc.vector.tensor_tensor(
                lg, ps4[:], b4s[:, None, :].to_broadcast((128, MG, C)), add_op
            )
            etile = spool.tile([128, MG, C], FP32, tag="etile")
            nc.scalar.activation(etile[:], lg, expf)
            nc.vector.tensor_reduce(
                esum_all[:, rg0 : rg0 + MG], etile[:],
                axis=mybir.AxisListType.X, op=add_op,
            )
            if sc == NCHUNK - 2:
                # Most of the epilogue hides under the last superchunk.
                softmax_epilogue(0, (NCHUNK - 1) * (SNB // 128))

        softmax_epilogue((NCHUNK - 1) * (SNB // 128), NRG)

    nc.compile()
    return nc


def _get_nc():
    global _CACHED_NC
    if _CACHED_NC is None:
        _CACHED_NC = build_nc()
    return _CACHED_NC


def make_in_maps(x, W1, b1, W2, b2, W3, b3, W4, b4):
    bf16 = ml_dtypes.bfloat16
    fp8 = ml_dtypes.float8_e4m3
    xq = np.asarray(x).astype(fp8)
    common = {
        "w1t": np.ascontiguousarray(np.asarray(W1).T.astype(fp8)),
        "w2t": np.ascontiguousarray(np.asarray(W2).T.astype(fp8)),
        "w3t": np.ascontiguousarray(np.asarray(W3).T.astype(fp8)),
        "w4t": np.ascontiguousarray(np.asarray(W4).T.astype(bf16)),
        "b1": np.asarray(b1).astype(np.float32),
        "b2": np.asarray(b2).astype(np.float32),
        "b3": np.asarray(b3).astype(np.float32),
        "b4r": np.tile(np.asarray(b4).astype(np.float32)[None, :], (128, 1)),
    }
    in_maps = []
    for i in range(N_CORES):
        shard = np.ascontiguousarray(xq[i * BC : (i + 1) * BC].T)  # [784, 8192]
        in_maps.append({"xt": shard, **common})
    return in_maps


def kernel(x, W1, b1, W2, b2, W3, b3, W4, b4):
    in_maps = make_in_maps(x, W1, b1, W2, b2, W3, b3, W4, b4)
    nc = _get_nc()
    res = run_bass_kernel_spmd(nc, in_maps, list(range(N_CORES)))
    out = np.concatenate(
        [res.results[i]["out"] for i in range(N_CORES)], axis=0
    ).astype(np.float32)
    return out



# revision 4
# speedup vs baseline: 1.0193x; 1.0193x over previous
"""Trainium2 Bass kernel: 4-layer MLP (784-512-512-512-10) + log_softmax.

Data-parallel over 8 NeuronCores: batch 65536 is split into 8 shards of
8192 rows; the ~1M-param weights are replicated on every core.

Layout: activations live on-chip transposed ([features, batch]) so every
layer's matmul is `out[of, nb] += W_lT[if, of].T @ h[if, nb]` with the
feature chunks on partitions.  Layers 1-3 run in fp8e4 with DoubleRow
(two 128-feature contraction chunks per matmul, fp32 PSUM accumulate);
layer 4 runs in bf16 with the batch flipped onto PSUM partitions so the
softmax reduces along the free dim.  Batch is processed in superchunks
of 1024 rows: each output-feature chunk accumulates two 512-column
halves into one 2-bank PSUM tile so a single 1024-wide op applies
bias+ReLU.

vs the first working version (170us):
 - layer-1's 784 = 3*256 (DoubleRow) + 16-row remainder: the remainder
   for all four output chunks now runs as four CONCURRENT row-tiled
   matmuls (tile_position=(32m,0), PE row groups), one span per batch
   half instead of eight full 512-cycle passes per superchunk.  The
   remainder x rows / W1 rows are host-replicated at partition offsets
   0/32/64/96 (xr / w1r tensors).
 - bias+ReLU PSUM drains alternate between ScalarE (activation) and
   DVE (tensor_scalar add-bias-then-max-0) so neither engine gates the
   PE's PSUM recycling.
 - all four PSUM pairs live in one pool (8 banks); layer 4 accumulates
   into a 40-column slice of a rotating pair.
 - weights load via four DMA queues (scalar/vector/gpsimd/sync) in
   split triggers so w1 lands ~9us and w2/w3 before layer 2/3 of the
   first superchunk; x superchunks prefetch 8-deep on the sync queue.
 - the activation table map is patched so Relu/Exp/Ln resolve to the
   single natural_log_exp_and_others table: one ACT_TABLE_LOAD at
   start-up (preloaded via a dummy activation) instead of four 1.3us
   loads, two of which sat on the critical tail.
 - output is written partition-major ([128, 64, 10] per core, host
   un-transposes) so the final DMA uses 2.5KB descriptors instead of
   40B ones (the old tail spent ~12us draining 40B descriptors).
 - PE warm-up trimmed to 16 N=128 matmuls that bridge the NEFF preamble
   to the first x/w chunk arrival, keeping the HAM clock gate busy.
"""

from contextlib import ExitStack

import ml_dtypes
import numpy as np

import concourse.bass as bass  # noqa: F401  (registers AP machinery)
from concourse import bacc, hw_specs, mybir
from concourse.bass_utils import run_bass_kernel_spmd
from concourse.tile import TileContext

BF16 = mybir.dt.bfloat16
FP32 = mybir.dt.float32
FP8 = mybir.dt.float8e4

N_CORES = 8
B = 65536
D0, H, C = 784, 512, 10
BC = B // N_CORES            # 8192 rows per core
NB = 512                     # matmul moving free dim / PSUM bank width
HB = 2                       # batch halves sharing one PSUM group
SNB = NB * HB                # 1024-row superchunk
NCHUNK = BC // SNB           # 8 superchunks
K0F = (D0 // 256) * 2        # 6 full 128-row contraction chunks in layer 1
K0R = D0 - K0F * 128         # 16 remainder rows
KH = H // 128                # 4 contraction chunks for hidden layers
NRG = BC // 128              # 64 row-groups of 128 rows per core

_CACHED_NC = None
_ACT_TABLES_PATCHED = False


def _patch_act_tables():
    """Make every activation resolve to the one table that holds
    relu+exp+ln together (natural_log_exp_and_others), so the kernel
    needs a single ACT_TABLE_LOAD instead of ping-ponging between the
    exp table and the ln table (1.28us per swap, two of which sat on
    the critical tail)."""
    global _ACT_TABLES_PATCHED
    if _ACT_TABLES_PATCHED:
        return
    _ACT_TABLES_PATCHED = True
    orig = hw_specs.get_activation_tables

    def patched(module_arch):
        tables = dict(orig(module_arch))
        keep = "natural_log_exp_and_others"
        if keep in tables:
            tables = {
                name: (funcs if name == keep else set())
                for name, funcs in tables.items()
            }
        return tables

    hw_specs.get_activation_tables = patched
    bacc.get_activation_tables = patched


def build_nc():
    _patch_act_tables()
    nc = bacc.Bacc(
        "TRN2",
        target_bir_lowering=False,
        debug=False,
        enable_asserts=False,
        num_devices=N_CORES,
    )
    xt_d = nc.declare_dram_parameter("xt", [K0F * 128, BC], FP8, isOutput=False)
    xr_d = nc.declare_dram_parameter("xr", [128, BC], FP8, isOutput=False)
    w1_d = nc.declare_dram_parameter("w1t", [K0F * 128, H], FP8, isOutput=False)
    w1r_d = nc.declare_dram_parameter("w1r", [128, 128], FP8, isOutput=False)
    w2_d = nc.declare_dram_parameter("w2t", [H, H], FP8, isOutput=False)
    w3_d = nc.declare_dram_parameter("w3t", [H, H], FP8, isOutput=False)
    w4_d = nc.declare_dram_parameter("w4r", [128, KH, C], BF16, isOutput=False)
    br_d = nc.declare_dram_parameter("brec", [128, 3 * KH + C], FP32, isOutput=False)
    # partition-major output: [p, rg, c] holds batch row rg*128 + p
    out_d = nc.declare_dram_parameter("out", [128, NRG, C], FP32, isOutput=True)

    expf = mybir.ActivationFunctionType.Exp
    reluf = mybir.ActivationFunctionType.Relu
    lnf = mybir.ActivationFunctionType.Ln
    add_op = mybir.AluOpType.add
    max_op = mybir.AluOpType.max
    sub_op = mybir.AluOpType.subtract
    drow = mybir.MatmulPerfMode.DoubleRow

    with TileContext(nc) as tc, ExitStack() as ctx:
        consts = ctx.enter_context(tc.tile_pool(name="consts", bufs=1))
        xpool = ctx.enter_context(tc.tile_pool(name="xp", bufs=8))
        xrpool = ctx.enter_context(tc.tile_pool(name="xrp", bufs=8))
        hpool = ctx.enter_context(tc.tile_pool(name="hp", bufs=3))
        spool = ctx.enter_context(tc.tile_pool(name="sp", bufs=4))
        pbig = ctx.enter_context(tc.tile_pool(name="pbig", bufs=4, space="PSUM"))

        # ---- resident weights/biases, spread over three DMA queues ----
        # (only SP/Activation/gpsimd can trigger DMAs)
        warm = consts.tile([128, NB], FP8, tag="warm")
        nc.vector.memset(warm[:], 1.0)
        # scalar queue: w1 in two halves so the first layer-1 matmuls
        # only wait for k-chunks 0-1; then the unified-ACT-table preload
        # (hides under the w1 transfer), then w2 halves and w4.
        w1 = consts.tile([128, K0F, H], FP8, tag="w1")
        nc.scalar.dma_start(
            w1[:, 0:2, :],
            w1_d[0 : 2 * 128, :].rearrange("(o p) n -> p o n", p=128),
        )
        nc.scalar.dma_start(
            w1[:, 2:K0F, :],
            w1_d[2 * 128 : K0F * 128, :].rearrange("(o p) n -> p o n", p=128),
        )
        scratch = consts.tile([128, 4], FP32, tag="scratch")
        nc.scalar.activation(
            scratch[:, 0:1], warm[:, 0:1], mybir.ActivationFunctionType.Relu
        )
        w2 = consts.tile([128, KH, H], FP8, tag="w2")
        nc.scalar.dma_start(
            w2[:, 0:2, :], w2_d[0:256, :].rearrange("(o p) n -> p o n", p=128)
        )
        nc.scalar.dma_start(
            w2[:, 2:4, :], w2_d[256:512, :].rearrange("(o p) n -> p o n", p=128)
        )
        w4 = consts.tile([128, KH, C], BF16, tag="w4")
        nc.scalar.dma_start(w4[:], w4_d[:])
        # gpsimd queue: layer-1 remainder weights + first x remainder +
        # biases + w3, then the remaining x remainder chunks.
        w1r = consts.tile([128, 128], FP8, tag="w1r")
        nc.gpsimd.dma_start(w1r[:], w1r_d[:])
        xrs = []
        xr0 = xrpool.tile([128, SNB], FP8, tag="xr", name="xr_0")
        nc.gpsimd.dma_start(xr0[:], xr_d[:, 0:SNB])
        xrs.append(xr0)
        brec = consts.tile([128, 3 * KH + C], FP32, tag="brec")
        nc.gpsimd.dma_start(brec[:], br_d[:])
        w3 = consts.tile([128, KH, H], FP8, tag="w3")
        nc.gpsimd.dma_start(
            w3[:], w3_d.rearrange("(o p) n -> p o n", p=128)
        )
        for sc in range(1, NCHUNK):
            xr = xrpool.tile([128, SNB], FP8, tag="xr", name=f"xr_{sc}")
            nc.gpsimd.dma_start(xr[:], xr_d[:, sc * SNB : (sc + 1) * SNB])
            xrs.append(xr)
        b4s = brec[:, 3 * KH : 3 * KH + C]

        # sync queue: x superchunks, 8-deep prefetch.  sc0 is split so
        # its first k-chunks land ~1us earlier.
        xts = []
        for sc in range(NCHUNK):
            b0 = sc * SNB
            xt = xpool.tile([128, K0F, SNB], FP8, tag="xt")
            if sc == 0:
                nc.sync.dma_start(
                    xt[:, 0:2, :],
                    xt_d[0:256, b0 : b0 + SNB].rearrange("(o p) n -> p o n", p=128),
                )
                nc.sync.dma_start(
                    xt[:, 2:K0F, :],
                    xt_d[256 : K0F * 128, b0 : b0 + SNB].rearrange(
                        "(o p) n -> p o n", p=128
                    ),
                )
            else:
                nc.sync.dma_start(
                    xt[:],
                    xt_d[:, b0 : b0 + SNB].rearrange("(o p) n -> p o n", p=128),
                )
            xts.append(xt)

        # ---- PE warm-up ----
        # 16 short matmuls bridge the NEFF preamble to the first x/w1
        # arrival so the HAM clock gate sees sustained PE activity.
        psw = pbig.tile([128, HB, NB], FP32, tag="ps", name="ps_warm")
        NWARM = 16
        for i in range(NWARM):
            nc.tensor.matmul(
                psw[:, i % 2, 0:128], lhsT=warm[:, 0:128], rhs=warm[:, 0:128],
                start=(i < 2), stop=(i >= NWARM - 2),
            )

        # Persistent softmax state for all 64 row-groups.
        logits_all = consts.tile([128, NRG, C], FP32, tag="logits_all")
        esum_all = consts.tile([128, NRG], FP32, tag="esum_all")
        lns_all = consts.tile([128, NRG], FP32, tag="lns_all")
        obuf = consts.tile([128, NRG, C], FP32, tag="obuf")

        def softmax_epilogue(rg0, rg1):
            # out = logits - ln(sum(exp(logits))) for row-groups [rg0, rg1)
            n = rg1 - rg0
            nc.scalar.activation(lns_all[:, rg0:rg1], esum_all[:, rg0:rg1], lnf)
            nc.vector.tensor_tensor(
                obuf[:, rg0:rg1, :], logits_all[:, rg0:rg1, :],
                lns_all[:, rg0:rg1, None].to_broadcast((128, n, C)), sub_op,
            )
            nc.sync.dma_start(out_d[:, rg0:rg1, :], obuf[:, rg0:rg1, :])

        def drain(engine_is_dve, dst, ps, bias_ap):
            # PSUM -> SBUF bias+ReLU, alternating engines so neither
            # gates PE PSUM recycling.
            if engine_is_dve:
                nc.vector.tensor_scalar(
                    dst, ps, bias_ap, 0.0, add_op, max_op
                )
            else:
                nc.scalar.activation(dst, ps, reluf, bias=bias_ap)

        for sc in range(NCHUNK):
            xt = xts[sc]
            xr = xrs[sc]

            # ---- Layer 1 [784 -> 512]: fp8, 3 DoubleRow passes + one
            # row-tiled concurrent span for the 16-row remainder. ----
            ps1 = [
                pbig.tile([128, HB, NB], FP32, tag="ps", name=f"ps1_{m}")
                for m in range(KH)
            ]
            for hb in range(HB):
                cs = slice(hb * NB, (hb + 1) * NB)
                for m in range(KH):
                    nc.tensor.matmul(
                        ps1[m][:, hb, :],
                        lhsT=w1r[32 * m : 32 * m + K0R, :],
                        rhs=xr[32 * m : 32 * m + K0R, cs],
                        start=True, stop=False,
                        tile_position=(32 * m, 0),
                    )
            h1p = [
                hpool.tile([128, 2, HB, NB], FP8, tag=f"h1p_{j}", name=f"h1p_{j}")
                for j in range(KH // 2)
            ]
            for m in range(KH):
                ms = slice(m * 128, (m + 1) * 128)
                for k in range(0, K0F, 2):
                    for hb in range(HB):
                        nc.tensor.matmul(
                            ps1[m][:, hb, :], lhsT=w1[:, k : k + 2, ms],
                            rhs=xt[:, k : k + 2, hb * NB : (hb + 1) * NB],
                            start=False, stop=(k == K0F - 2), perf_mode=drow,
                        )
                drain(
                    m % 2 == 1,
                    h1p[m // 2][:, m % 2, :, :], ps1[m][:], brec[:, m : m + 1],
                )

            # ---- Layer 2 [512 -> 512]: fp8 DoubleRow over chunk pairs ----
            h2p = [
                hpool.tile([128, 2, HB, NB], FP8, tag=f"h2p_{j}", name=f"h2p_{j}")
                for j in range(KH // 2)
            ]
            for m in range(KH):
                ps = pbig.tile([128, HB, NB], FP32, tag="ps")
                ms = slice(m * 128, (m + 1) * 128)
                for j in range(KH // 2):
                    for hb in range(HB):
                        nc.tensor.matmul(
                            ps[:, hb, :], lhsT=w2[:, 2 * j : 2 * j + 2, ms],
                            rhs=h1p[j][:, :, hb, :],
                            start=(j == 0), stop=(j == KH // 2 - 1),
                            perf_mode=drow,
                        )
                drain(
                    m % 2 == 1,
                    h2p[m // 2][:, m % 2, :, :], ps[:], brec[:, KH + m : KH + m + 1],
                )

            # ---- Layer 3 [512 -> 512]: fp8 DoubleRow in, bf16 out ----
            h3 = [
                hpool.tile([128, HB, NB], BF16, tag=f"h3_{m}", name=f"h3_{m}")
                for m in range(KH)
            ]
            for m in range(KH):
                ps = pbig.tile([128, HB, NB], FP32, tag="ps")
                ms = slice(m * 128, (m + 1) * 128)
                for j in range(KH // 2):
                    for hb in range(HB):
                        nc.tensor.matmul(
                            ps[:, hb, :], lhsT=w3[:, 2 * j : 2 * j + 2, ms],
                            rhs=h2p[j][:, :, hb, :],
                            start=(j == 0), stop=(j == KH // 2 - 1),
                            perf_mode=drow,
                        )
                drain(
                    m % 2 == 1,
                    h3[m][:], ps[:], brec[:, 2 * KH + m : 2 * KH + m + 1],
                )

            # ---- Layer 4 [512 -> 10], bf16, batch flipped onto PSUM
            # partitions.  All 8 row-groups accumulate into 40-column
            # slices of one rotating PSUM pair. ----
            MG = SNB // 128
            rg0 = sc * MG
            ps4 = pbig.tile([128, HB, NB], FP32, tag="ps", name="ps4")
            for hb in range(HB):
                for mm in range(NB // 128):
                    ms = slice(mm * 128, (mm + 1) * 128)
                    od = slice(mm * C, (mm + 1) * C)
                    for k in range(KH):
                        nc.tensor.matmul(
                            ps4[:, hb, od], lhsT=h3[k][:, hb, ms], rhs=w4[:, k, :],
                            start=(k == 0), stop=(k == KH - 1),
                        )
            for hb in range(HB):
                lg = logits_all[:, rg0 + 4 * hb : rg0 + 4 * hb + 4, :]
                nc.vector.tensor_tensor(
                    lg,
                    ps4[:, hb, 0 : 4 * C].rearrange("p (r c) -> p r c", c=C),
                    b4s[:, None, :].to_broadcast((128, 4, C)), add_op,
                )
            etile = spool.tile([128, MG, C], FP32, tag="etile")
            nc.scalar.activation(etile[:], logits_all[:, rg0 : rg0 + MG, :], expf)
            nc.vector.tensor_reduce(
                esum_all[:, rg0 : rg0 + MG], etile[:],
                axis=mybir.AxisListType.X, op=add_op,
            )
            if sc == NCHUNK - 2:
                # Most of the epilogue hides under the last superchunk.
                softmax_epilogue(0, (NCHUNK - 1) * (SNB // 128))

        softmax_epilogue((NCHUNK - 1) * (SNB // 128), NRG)

    nc.compile()
    return nc


def _get_nc():
    global _CACHED_NC
    if _CACHED_NC is None:
        _CACHED_NC = build_nc()
    return _CACHED_NC


def make_in_maps(x, W1, b1, W2, b2, W3, b3, W4, b4):
    bf16 = ml_dtypes.bfloat16
    fp8 = ml_dtypes.float8_e4m3
    xq = np.asarray(x).astype(fp8)
    w1t = np.asarray(W1).T.astype(fp8)             # [784, 512]
    w1r = np.zeros((128, 128), dtype=fp8)
    for m in range(KH):
        w1r[32 * m : 32 * m + K0R, :] = w1t[K0F * 128 :, m * 128 : (m + 1) * 128]
    w4r = (
        np.asarray(W4).T.astype(bf16)              # [512, 10]
        .reshape(KH, 128, C).transpose(1, 0, 2)    # [128, 4, 10]
    )
    brec = np.hstack(
        [
            np.asarray(b1).astype(np.float32).reshape(KH, 128).T,
            np.asarray(b2).astype(np.float32).reshape(KH, 128).T,
            np.asarray(b3).astype(np.float32).reshape(KH, 128).T,
            np.tile(np.asarray(b4).astype(np.float32)[None, :], (128, 1)),
        ]
    )
    common = {
        "w1t": np.ascontiguousarray(w1t[0 : K0F * 128]),
        "w1r": w1r,
        "w2t": np.ascontiguousarray(np.asarray(W2).T.astype(fp8)),
        "w3t": np.ascontiguousarray(np.asarray(W3).T.astype(fp8)),
        "w4r": np.ascontiguousarray(w4r),
        "brec": np.ascontiguousarray(brec),
    }
    in_maps = []
    for i in range(N_CORES):
        shard = np.ascontiguousarray(xq[i * BC : (i + 1) * BC].T)  # [784, 8192]
        xr = np.zeros((128, BC), dtype=fp8)
        for m in range(KH):
            xr[32 * m : 32 * m + K0R, :] = shard[K0F * 128 :]
        in_maps.append(
            {"xt": np.ascontiguousarray(shard[0 : K0F * 128]), "xr": xr, **common}
        )
    return in_maps


def gather_out(res):
    # out is [128, 64, 10] partition-major per core: row rg*128 + p
    # lives at [p, rg, :].
    return np.concatenate(
        [
            np.asarray(res.results[i]["out"])
            .transpose(1, 0, 2)
            .reshape(BC, C)
            for i in range(N_CORES)
        ],
        axis=0,
    ).astype(np.float32)


def kernel(x, W1, b1, W2, b2, W3, b3, W4, b4):
    in_maps = make_in_maps(x, W1, b1, W2, b2, W3, b3, W4, b4)
    nc = _get_nc()
    res = run_bass_kernel_spmd(nc, in_maps, list(range(N_CORES)))
    return gather_out(res)


# revision 5
# speedup vs baseline: 1.1371x; 1.1156x over previous
"""Trainium2 Bass kernel: 4-layer MLP (784-512-512-512-10) + log_softmax.

Data-parallel over 8 NeuronCores: batch 65536 is split into 8 shards of
8192 rows; the ~1M-param weights are replicated on every core.

Layout: activations live on-chip transposed ([features, batch]) so every
layer's matmul is `out[of, nb] += W_lT[if, of].T @ h[if, nb]` with the
feature chunks on partitions.  Layers 1-3 run in fp8e4 with DoubleRow
(fp32 PSUM accumulate); layer 4 runs in bf16 with the batch flipped onto
PSUM partitions so the softmax reduces along the free dim.  Batch is
processed in superchunks of 1024 rows; each output-feature chunk
accumulates two 512-column halves into one 2-bank PSUM pair so a single
1024-wide op applies bias+ReLU.

Key structure (vs the 170us first working version):
 - layer 1's K=784 is split as 8 chunks of 98 partitions, so all four
   contraction passes are uniform DoubleRow (no 16-row remainder pass
   that costs a full 512-cycle stream).
 - layer-1 passes run k-pair-major for kp0/kp1, then per-m kp2+kp3 with
   the drain immediately after, so h1 chunks are ready the moment the
   PE finishes layer 1 (no drain-latency stall into layer 2).
 - bias+ReLU PSUM drains alternate ScalarE (activation) / DVE
   (tensor_scalar add-bias-then-max-0) so neither engine gates PSUM
   recycling.
 - layer 4 of superchunk N is emitted in the middle of superchunk N+1's
   layer-1 block: its 32 tiny (N=10) matmuls are then surrounded by
   dense 512-cycle DoubleRow streams, which keeps the PE HAM activity
   window busy (a contiguous low-activity stretch re-throttles the PE
   clock to 1.2GHz for 3.4us+ — measured as once-per-superchunk K=4/8
   oscillation costing ~19us).
 - the activation-table map is patched so Relu/Exp/Ln all resolve to
   the single natural_log_exp_and_others table: one ACT_TABLE_LOAD at
   start-up (preloaded via a dummy activation) instead of four 1.28us
   loads, two of which sat on the critical tail.
 - output is written partition-major ([128, 64, 10] per core, host
   un-transposes) so the final DMA uses KB-scale descriptors instead of
   40B ones (the old tail spent ~12us draining 40B descriptors).
 - weights + the first superchunk's x are split over the three DMA
   trigger queues (scalar/sync/gpsimd) in arrival-order-matched pieces;
   later x superchunks prefetch 8-deep on the sync queue.
 - PE warm-up: a few N=512 matmuls bridge the NEFF preamble to the
   first x/w chunk arrival so the HAM clock gate sees sustained
   activity (LDW-dominated warm-ups don't count as busy).
"""

from contextlib import ExitStack

import ml_dtypes
import numpy as np

import concourse.bass as bass  # noqa: F401  (registers AP machinery)
from concourse import bacc, hw_specs, mybir
from concourse.bass_utils import run_bass_kernel_spmd
from concourse.tile import TileContext

BF16 = mybir.dt.bfloat16
FP32 = mybir.dt.float32
FP8 = mybir.dt.float8e4

N_CORES = 8
B = 65536
D0, H, C = 784, 512, 10
BC = B // N_CORES            # 8192 rows per core
NB = 512                     # matmul moving free dim / PSUM bank width
HB = 2                       # batch halves sharing one PSUM group
SNB = NB * HB                # 1024-row superchunk
NCHUNK = BC // SNB           # 8 superchunks
KP = 98                      # layer-1 contraction chunk height (8*98=784)
K1 = D0 // KP                # 8 layer-1 chunks -> 4 DoubleRow passes
KH = H // 128                # 4 contraction chunks for hidden layers
NRG = BC // 128              # 64 row-groups of 128 rows per core
MG = SNB // 128              # 8 row-groups per superchunk

_CACHED_NC = None
_ACT_TABLES_PATCHED = False


def _patch_act_tables():
    """Make every activation resolve to the one table that holds
    relu+exp+ln together (natural_log_exp_and_others), so the kernel
    needs a single ACT_TABLE_LOAD instead of ping-ponging between the
    exp table and the ln table (1.28us per swap)."""
    global _ACT_TABLES_PATCHED
    if _ACT_TABLES_PATCHED:
        return
    _ACT_TABLES_PATCHED = True
    orig = hw_specs.get_activation_tables

    def patched(module_arch):
        tables = dict(orig(module_arch))
        keep = "natural_log_exp_and_others"
        if keep in tables:
            tables = {
                name: (funcs if name == keep else set())
                for name, funcs in tables.items()
            }
        return tables

    hw_specs.get_activation_tables = patched
    bacc.get_activation_tables = patched


def build_nc():
    _patch_act_tables()
    nc = bacc.Bacc(
        "TRN2",
        target_bir_lowering=False,
        debug=False,
        enable_asserts=False,
        num_devices=N_CORES,
    )
    xt_d = nc.declare_dram_parameter("xt", [D0, BC], FP8, isOutput=False)
    w1_d = nc.declare_dram_parameter("w1t", [D0, H], FP8, isOutput=False)
    w2_d = nc.declare_dram_parameter("w2t", [H, H], FP8, isOutput=False)
    w3_d = nc.declare_dram_parameter("w3t", [H, H], FP8, isOutput=False)
    w4_d = nc.declare_dram_parameter("w4r", [128, KH, C], BF16, isOutput=False)
    br_d = nc.declare_dram_parameter("brec", [128, 3 * KH + C], FP32, isOutput=False)
    # partition-major output: [p, rg, c] holds batch row rg*128 + p
    out_d = nc.declare_dram_parameter("out", [128, NRG, C], FP32, isOutput=True)

    expf = mybir.ActivationFunctionType.Exp
    reluf = mybir.ActivationFunctionType.Relu
    lnf = mybir.ActivationFunctionType.Ln
    add_op = mybir.AluOpType.add
    max_op = mybir.AluOpType.max
    sub_op = mybir.AluOpType.subtract
    drow = mybir.MatmulPerfMode.DoubleRow

    def x_rows(k0, k1_):
        # [98, k, SNB/...] view of rows k0*98 .. k1_*98
        return slice(k0 * KP, k1_ * KP)

    with TileContext(nc) as tc, ExitStack() as ctx:
        consts = ctx.enter_context(tc.tile_pool(name="consts", bufs=1))
        xpool = ctx.enter_context(tc.tile_pool(name="xp", bufs=8))
        hpool = ctx.enter_context(tc.tile_pool(name="hp", bufs=3))
        spool = ctx.enter_context(tc.tile_pool(name="sp", bufs=4))
        pbig = ctx.enter_context(tc.tile_pool(name="pbig", bufs=4, space="PSUM"))

        warm = consts.tile([128, NB], FP8, tag="warm")
        nc.vector.memset(warm[:], 1.0)

        # ---- resident weights/biases over the three DMA trigger queues,
        # ordered to match first-use times ----
        # scalar queue: w1 halves, ACT-table preload, x0 tail, w4.
        w1 = consts.tile([KP, K1, H], FP8, tag="w1")
        nc.scalar.dma_start(
            w1[:, 0:4, :],
            w1_d[x_rows(0, 4), :].rearrange("(k p) n -> p k n", p=KP),
        )
        nc.scalar.dma_start(
            w1[:, 4:8, :],
            w1_d[x_rows(4, 8), :].rearrange("(k p) n -> p k n", p=KP),
        )
        scratch = consts.tile([128, 4], FP32, tag="scratch")
        nc.scalar.activation(scratch[:, 0:1], warm[:, 0:1], reluf)
        xt0 = xpool.tile([KP, K1, SNB], FP8, tag="xt", name="xt_0")
        nc.scalar.dma_start(
            xt0[:, 6:8, :],
            xt_d[x_rows(6, 8), 0:SNB].rearrange("(k p) n -> p k n", p=KP),
        )
        w4 = consts.tile([128, KH, C], BF16, tag="w4")
        nc.scalar.dma_start(w4[:], w4_d[:])
        # gpsimd queue: biases, x0 middle, w2 halves, w3.
        brec = consts.tile([128, 3 * KH + C], FP32, tag="brec")
        nc.gpsimd.dma_start(brec[:], br_d[:])
        nc.gpsimd.dma_start(
            xt0[:, 4:6, :],
            xt_d[x_rows(4, 6), 0:SNB].rearrange("(k p) n -> p k n", p=KP),
        )
        w2 = consts.tile([128, KH, H], FP8, tag="w2")
        nc.gpsimd.dma_start(
            w2[:, 0:2, :], w2_d[0:256, :].rearrange("(o p) n -> p o n", p=128)
        )
        nc.gpsimd.dma_start(
            w2[:, 2:4, :], w2_d[256:512, :].rearrange("(o p) n -> p o n", p=128)
        )
        w3 = consts.tile([128, KH, H], FP8, tag="w3")
        nc.gpsimd.dma_start(w3[:], w3_d.rearrange("(o p) n -> p o n", p=128))
        b4s = brec[:, 3 * KH : 3 * KH + C]
        # sync queue: x0 head in two pieces, then x1..x7, 8-deep prefetch.
        nc.sync.dma_start(
            xt0[:, 0:2, :],
            xt_d[x_rows(0, 2), 0:SNB].rearrange("(k p) n -> p k n", p=KP),
        )
        nc.sync.dma_start(
            xt0[:, 2:4, :],
            xt_d[x_rows(2, 4), 0:SNB].rearrange("(k p) n -> p k n", p=KP),
        )
        xts = [xt0]
        for sc in range(1, NCHUNK):
            b0 = sc * SNB
            xt = xpool.tile([KP, K1, SNB], FP8, tag="xt", name=f"xt_{sc}")
            nc.sync.dma_start(
                xt[:],
                xt_d[:, b0 : b0 + SNB].rearrange("(k p) n -> p k n", p=KP),
            )
            xts.append(xt)

        # ---- PE warm-up: N=512 streams keep the HAM activity counter
        # fed from the end of the NEFF preamble to first-data. ----
        psw = pbig.tile([128, HB, NB], FP32, tag="ps", name="ps_warm")
        NWARM = 5
        for i in range(NWARM):
            nc.tensor.matmul(
                psw[:, i % 2, :], lhsT=warm[:, 0:128], rhs=warm[:],
                start=(i < 2), stop=(i >= NWARM - 2),
            )

        # Persistent softmax state for all 64 row-groups.
        logits_all = consts.tile([128, NRG, C], FP32, tag="logits_all")
        esum_all = consts.tile([128, NRG], FP32, tag="esum_all")
        lns_all = consts.tile([128, NRG], FP32, tag="lns_all")
        obuf = consts.tile([128, NRG, C], FP32, tag="obuf")

        def softmax_epilogue(rg0, rg1):
            # out = logits - ln(sum(exp(logits))) for row-groups [rg0, rg1)
            n = rg1 - rg0
            nc.scalar.activation(lns_all[:, rg0:rg1], esum_all[:, rg0:rg1], lnf)
            nc.vector.tensor_tensor(
                obuf[:, rg0:rg1, :], logits_all[:, rg0:rg1, :],
                lns_all[:, rg0:rg1, None].to_broadcast((128, n, C)), sub_op,
            )
            nc.sync.dma_start(out_d[:, rg0:rg1, :], obuf[:, rg0:rg1, :])

        def drain(engine_is_dve, dst, ps, bias_ap):
            # PSUM -> SBUF bias+ReLU, alternating engines so neither
            # gates PE PSUM recycling.
            if engine_is_dve:
                nc.vector.tensor_scalar(dst, ps, bias_ap, 0.0, add_op, max_op)
            else:
                nc.scalar.activation(dst, ps, reluf, bias=bias_ap)

        h3s = [None] * NCHUNK

        def l4_block(sc):
            # Layer 4 [512 -> 10] for superchunk sc, bf16, batch flipped
            # onto PSUM partitions; accumulates into 40-column slices of
            # one rotating PSUM pair, then logits/exp/sum-exp.
            h3 = h3s[sc]
            rg0 = sc * MG
            ps4 = pbig.tile([128, HB, NB], FP32, tag="ps", name=f"ps4_{sc}")
            for hb in range(HB):
                for mm in range(NB // 128):
                    ms = slice(mm * 128, (mm + 1) * 128)
                    od = slice(mm * C, (mm + 1) * C)
                    for k in range(KH):
                        nc.tensor.matmul(
                            ps4[:, hb, od], lhsT=h3[k][:, hb, ms],
                            rhs=w4[:, k, :],
                            start=(k == 0), stop=(k == KH - 1),
                        )
            for hb in range(HB):
                lg = logits_all[:, rg0 + 4 * hb : rg0 + 4 * hb + 4, :]
                nc.vector.tensor_tensor(
                    lg,
                    ps4[:, hb, 0 : 4 * C].rearrange("p (r c) -> p r c", c=C),
                    b4s[:, None, :].to_broadcast((128, 4, C)), add_op,
                )
            etile = spool.tile([128, MG, C], FP32, tag="etile")
            nc.scalar.activation(etile[:], logits_all[:, rg0 : rg0 + MG, :], expf)
            nc.vector.tensor_reduce(
                esum_all[:, rg0 : rg0 + MG], etile[:],
                axis=mybir.AxisListType.X, op=add_op,
            )

        for sc in range(NCHUNK):
            xt = xts[sc]

            # ---- Layer 1 [784 -> 512]: 4 uniform fp8 DoubleRow passes
            # (8 chunks of 98).  kp0/kp1 k-major; kp2/kp3 m-major with
            # the drain immediately after each m. ----
            ps1 = [
                pbig.tile([128, HB, NB], FP32, tag="ps", name=f"ps1_{m}")
                for m in range(KH)
            ]
            for kp in range(2):
                for m in range(KH):
                    ms = slice(m * 128, (m + 1) * 128)
                    for hb in range(HB):
                        nc.tensor.matmul(
                            ps1[m][:, hb, :], lhsT=w1[:, 2 * kp : 2 * kp + 2, ms],
                            rhs=xt[:, 2 * kp : 2 * kp + 2, hb * NB : (hb + 1) * NB],
                            start=(kp == 0), stop=False, perf_mode=drow,
                        )

            # layer 4 of the previous superchunk sits here, surrounded
            # by dense DoubleRow streams (keeps the HAM window busy).
            if sc > 0:
                l4_block(sc - 1)
            if sc == NCHUNK - 1:
                # most of the softmax epilogue hides under the last
                # superchunk's compute.
                softmax_epilogue(0, (NCHUNK - 1) * MG)

            h1p = [
                hpool.tile([128, 2, HB, NB], FP8, tag=f"h1p_{j}", name=f"h1p_{j}")
                for j in range(KH // 2)
            ]
            for m in range(KH):
                ms = slice(m * 128, (m + 1) * 128)
                for kp in range(2, 4):
                    for hb in range(HB):
                        nc.tensor.matmul(
                            ps1[m][:, hb, :], lhsT=w1[:, 2 * kp : 2 * kp + 2, ms],
                            rhs=xt[:, 2 * kp : 2 * kp + 2, hb * NB : (hb + 1) * NB],
                            start=False, stop=(kp == 3), perf_mode=drow,
                        )
                drain(
                    m % 2 == 1,
                    h1p[m // 2][:, m % 2, :, :], ps1[m][:], brec[:, m : m + 1],
                )

            # ---- Layer 2 [512 -> 512]: fp8 DoubleRow over chunk pairs ----
            h2p = [
                hpool.tile([128, 2, HB, NB], FP8, tag=f"h2p_{j}", name=f"h2p_{j}")
                for j in range(KH // 2)
            ]
            for m in range(KH):
                ps = pbig.tile([128, HB, NB], FP32, tag="ps")
                ms = slice(m * 128, (m + 1) * 128)
                for j in range(KH // 2):
                    for hb in range(HB):
                        nc.tensor.matmul(
                            ps[:, hb, :], lhsT=w2[:, 2 * j : 2 * j + 2, ms],
                            rhs=h1p[j][:, :, hb, :],
                            start=(j == 0), stop=(j == KH // 2 - 1),
                            perf_mode=drow,
                        )
                drain(
                    m % 2 == 1,
                    h2p[m // 2][:, m % 2, :, :], ps[:], brec[:, KH + m : KH + m + 1],
                )

            # ---- Layer 3 [512 -> 512]: fp8 DoubleRow in, bf16 out ----
            h3 = [
                hpool.tile([128, HB, NB], BF16, tag=f"h3_{m}", name=f"h3_{m}")
                for m in range(KH)
            ]
            h3s[sc] = h3
            for m in range(KH):
                ps = pbig.tile([128, HB, NB], FP32, tag="ps")
                ms = slice(m * 128, (m + 1) * 128)
                for j in range(KH // 2):
                    for hb in range(HB):
                        nc.tensor.matmul(
                            ps[:, hb, :], lhsT=w3[:, 2 * j : 2 * j + 2, ms],
                            rhs=h2p[j][:, :, hb, :],
                            start=(j == 0), stop=(j == KH // 2 - 1),
                            perf_mode=drow,
                        )
                drain(
                    m % 2 == 1,
                    h3[m][:], ps[:], brec[:, 2 * KH + m : 2 * KH + m + 1],
                )

        l4_block(NCHUNK - 1)
        softmax_epilogue((NCHUNK - 1) * MG, NRG)

    nc.compile()
    return nc


def _get_nc():
    global _CACHED_NC
    if _CACHED_NC is None:
        _CACHED_NC = build_nc()
    return _CACHED_NC


def make_in_maps(x, W1, b1, W2, b2, W3, b3, W4, b4):
    bf16 = ml_dtypes.bfloat16
    fp8 = ml_dtypes.float8_e4m3
    xq = np.asarray(x).astype(fp8)
    w4r = (
        np.asarray(W4).T.astype(bf16)              # [512, 10]
        .reshape(KH, 128, C).transpose(1, 0, 2)    # [128, 4, 10]
    )
    brec = np.hstack(
        [
            np.asarray(b1).astype(np.float32).reshape(KH, 128).T,
            np.asarray(b2).astype(np.float32).reshape(KH, 128).T,
            np.asarray(b3).astype(np.float32).reshape(KH, 128).T,
            np.tile(np.asarray(b4).astype(np.float32)[None, :], (128, 1)),
        ]
    )
    common = {
        "w1t": np.ascontiguousarray(np.asarray(W1).T.astype(fp8)),
        "w2t": np.ascontiguousarray(np.asarray(W2).T.astype(fp8)),
        "w3t": np.ascontiguousarray(np.asarray(W3).T.astype(fp8)),
        "w4r": np.ascontiguousarray(w4r),
        "brec": np.ascontiguousarray(brec),
    }
    in_maps = []
    for i in range(N_CORES):
        shard = np.ascontiguousarray(xq[i * BC : (i + 1) * BC].T)  # [784, 8192]
        in_maps.append({"xt": shard, **common})
    return in_maps


def gather_out(res):
    # out is [128, 64, 10] partition-major per core: row rg*128 + p
    # lives at [p, rg, :].
    return np.concatenate(
        [
            np.asarray(res.results[i]["out"])
            .transpose(1, 0, 2)
            .reshape(BC, C)
            for i in range(N_CORES)
        ],
        axis=0,
    ).astype(np.float32)


def kernel(x, W1, b1, W2, b2, W3, b3, W4, b4):
    in_maps = make_in_maps(x, W1, b1, W2, b2, W3, b3, W4, b4)
    nc = _get_nc()
    res = run_bass_kernel_spmd(nc, in_maps, list(range(N_CORES)))
    return gather_out(res)
